# revision 41
# baseline (speedup 1.0000x reference)
"""ATMASKDGCNN Trainium2 kernel.

Data-parallel over batch: 8 samples -> 8 NeuronCores, one sample per core.
Each core runs an identical Bass program on its own sample; no collectives.

Math (validated vs reference in fp64):
  EdgeConv(x; W, bn) = lrelu(max_k p[idx[n,k]] + q[n])
    with p = x @ (s*Wd)^T, q = x @ (s*(Wc-Wd))^T + (beta - s*mean),
    Wd = W[:, :C], Wc = W[:, C:], s = gamma*rsqrt(var+eps).
  KNN ranking by r[n,m] = x_n . x_m - |x_m|^2/2  (row-monotone equiv of -d^2/2)
  pd[n,m] = 2*r[n,m] - |x_n|^2  (used for eu in the L1 mask branch).
  mask  = sigmoid(|a| * max_k(sign(a)*y2) + b)  (bn2m affine folded; sign in wm2)
  h     = mask * relu(bn5(cat @ W5^T))   (relu(lrelu(z)) == relu(z); mask>0)
  head: plain affine-folded MLP per sample.
"""

import os
import numpy as np

N = 2048
K = 20
NT = N // 128  # 16 point-tiles
BN_EPS = 1e-5
NEG_BIG = -3.0e38
USE_F32R = False

_CACHE = {}


# --------------------------------------------------------------------------
# host-side weight preprocessing
# --------------------------------------------------------------------------
def _bn_affine(p):
    g, b, m, v = [np.asarray(t, np.float64) for t in p]
    s = g / np.sqrt(v + BN_EPS)
    return s, b - s * m


def prepare_weights(params):
    f32 = lambda a: np.ascontiguousarray(a, np.float32)
    W = {k: np.asarray(v, np.float64) for k, v in params.items() if not isinstance(v, tuple)}
    s1, t1 = _bn_affine(params["bn1"]); s2, t2 = _bn_affine(params["bn2"])
    s3, t3 = _bn_affine(params["bn3"]); s4, t4 = _bn_affine(params["bn4"])
    s5, t5 = _bn_affine(params["bn5"]); sm1, tm1 = _bn_affine(params["bnm1"])
    sm2, tm2 = _bn_affine(params["bnm2"]); s6, t6 = _bn_affine(params["bn6"])
    s7, t7 = _bn_affine(params["bn7"])

    out = {}
    # L1: p rows [p0(64) | tm_nb(6) | pad(2)]
    W1 = W["W1"]; Wm1 = W["Wm1"]
    P1 = np.zeros((3, 128)); P1[:, :64] = (s1[:, None] * W1[:, :3]).T
    P1[:, 64:70] = (sm1[:, None] * (Wm1[:, 0:3] + Wm1[:, 6:9])).T
    out["wP1"] = f32(P1)
    Q1 = np.zeros((3, 70)); Q1[:, :64] = (s1[:, None] * (W1[:, 3:6] - W1[:, :3])).T
    Q1[:, 64:70] = (sm1[:, None] * (Wm1[:, 3:6] - Wm1[:, 0:3])).T
    out["wQ1"] = f32(Q1)
    B1 = np.zeros((1, 70)); B1[0, :64] = t1; B1[0, 64:70] = tm1
    out["bQ1"] = f32(B1)

    for i, (Wk, sk, tk, C) in enumerate(
        [("W2", s2, t2, 64), ("W3", s3, t3, 64), ("W4", s4, t4, 128)], start=2
    ):
        Wi = W[Wk]
        out[f"wP{i}"] = f32((sk[:, None] * Wi[:, :C]).T)
        out[f"wQ{i}"] = f32((sk[:, None] * (Wi[:, C:] - Wi[:, :C])).T)
        out[f"bQ{i}"] = f32(tk[None, :])

    # mask branch constants
    a = sm2[0]; bm = tm2[0]; sgn = 1.0 if a >= 0 else -1.0
    out["w9pm"] = f32(np.broadcast_to((sm1 * Wm1[:, 9])[None, :], (128, 6)))
    out["wm2pm"] = f32(np.broadcast_to((sgn * W["Wm2"][0])[None, :], (128, 6)))
    out["_mask_scale"] = float(abs(a)); out["_mask_bias"] = float(bm)

    # W5 (s5 folded), bias via mask-row matmul
    out["W5p"] = f32((s5[:, None] * W["W5"]).T)        # [512, 1024]
    out["t5row"] = f32(t5[None, :])                    # [1, 1024]

    # head L1: g layout col j*128+p: j<8 -> max of h[j*128+p], j>=8 -> sum/2048
    L1 = W["L1"]  # [512, 2048]
    hL1 = np.empty((2048, 512))
    for j in range(16):
        for_cols = np.arange(128) + (j * 128 if j < 8 else 1024 + (j - 8) * 128)
        blk = L1[:, for_cols].T * s6[None, :]
        hL1[j * 128:(j + 1) * 128] = blk
    out["hL1"] = f32(hL1)
    out["t6row"] = f32(t6[None, :])
    out["hL2"] = f32((s7[:, None] * W["L2"]).T)        # [512, 256]
    out["bL2"] = f32((s7 * W["b2"] + t7)[None, :])
    out["hL3"] = f32(W["L3"].T)                        # [256, 40]
    out["bL3"] = f32(W["b3"][None, :])
    out["ident"] = f32(np.eye(128))
    return pack_weights(out)


# pack layout tables: (name, rows, cols). Folded entries in B are pre-folded
# host-side into [128, cols].
PACK_A = [
    ("wP1", 3, 128), ("wQ1", 3, 70), ("bQ1", 1, 70),
    ("wP2", 64, 64), ("wQ2", 64, 64), ("bQ2", 1, 64),
    ("wP3", 64, 128), ("wQ3", 64, 128), ("bQ3", 1, 128),
    ("wP4", 128, 256), ("wQ4", 128, 256), ("bQ4", 1, 256),
    ("w9pm", 128, 6), ("wm2pm", 128, 6), ("ident", 128, 128),
]
PACK_A += [("t6row", 1, 512), ("bL2", 1, 256), ("bL3", 1, 40)]
# fp16 head weights, folded host-side into [128, cols]
PACK_C = [
    ("W5p", 128, 4096), ("hL1", 128, 8192), ("hL2", 128, 1024),
    ("hL3", 128, 80), ("t5row", 1, 1024),
]
FA = sum(c for _, _, c in PACK_A)
FC = sum(c for _, _, c in PACK_C)


def _fold(a, p=128):
    k = a.shape[0] // p
    return a.reshape(k, p, a.shape[1]).transpose(1, 0, 2).reshape(p, -1)


def pack_weights(w):
    packA = np.zeros((128, FA), np.float32)
    off = 0
    for nm, rows, cols in PACK_A:
        packA[0:rows, off:off + cols] = w[nm]
        off += cols
    packC = np.zeros((128, FC), np.float16)
    off = 0
    for nm, rows, cols in PACK_C:
        a = w[nm]
        if a.shape[0] > 128:
            a = _fold(a)
        packC[0:rows, off:off + cols] = a.reshape(rows, cols).astype(np.float16)
        off += cols
    return {"wpackA": packA, "wpackC": packC,
            "_mask_scale": w["_mask_scale"], "_mask_bias": w["_mask_bias"]}


# --------------------------------------------------------------------------
# device program
# --------------------------------------------------------------------------
def build_program(mask_scale, mask_bias, debug=False):
    import concourse.bass as bass
    import concourse.bacc as bacc
    import concourse.mybir as mybir
    import concourse.tile as tile
    from concourse.bass import IndirectOffsetOnAxis

    FP = mybir.dt.float32
    U32 = mybir.dt.uint32
    U16 = mybir.dt.uint16
    I16 = mybir.dt.int16
    ALU = mybir.AluOpType
    AXT = mybir.AxisListType
    ACTF = mybir.ActivationFunctionType

    nc = bacc.Bacc("TRN2", target_bir_lowering=False, debug=False)

    # ---- I/O ----
    x_in = nc.dram_tensor("x", [3, N], FP, kind="ExternalInput")
    packA_dr = nc.dram_tensor("wpackA", [128, FA], FP, kind="ExternalInput")
    packC_dr = nc.dram_tensor("wpackC", [128, FC], mybir.dt.float16, kind="ExternalInput")
    out_dram = nc.dram_tensor("out", [1, 40], FP, kind="ExternalOutput")

    sdram = nc.dram_tensor("sdram", [1, N], mybir.dt.float32, kind="Internal")
    idxd = nc.dram_tensor("idxd", [NT * 128, 20], mybir.dt.uint16, kind="Internal")
    mdram = nc.dram_tensor("mdram", [1, N], mybir.dt.float32, kind="Internal")
    pdram = [
        nc.dram_tensor("pd1", [N, 128], FP, kind="Internal"),
        nc.dram_tensor("pd2", [N, 64], FP, kind="Internal"),
        nc.dram_tensor("pd3", [N, 128], FP, kind="Internal"),
        nc.dram_tensor("pd4", [N, 256], FP, kind="Internal"),
    ]
    dbg = {}
    if debug:
        for nm, shp in [("d_x1T", (64, N)), ("d_x2T", (64, N)), ("d_x3T", (128, N)),
                        ("d_x4Ta", (128, N)), ("d_x4Tb", (128, N)),
                        ("d_mask", (128, 16)), ("d_g", (128, 16)),
                        ("d_idx1", (128, 24)), ("d_r1", (128, N))]:
            dbg[nm] = nc.dram_tensor(nm, list(shp), FP if not nm.startswith("d_idx") else mybir.dt.uint16,
                                     kind="ExternalOutput")

    with tile.TileContext(nc) as tc:
        with tc.tile_pool(name="persist", bufs=1) as pp, \
             tc.tile_pool(name="psum", bufs=1, space="PSUM") as psp:

            # ---- load layer weights (single packed DMA) ----
            wsb = {}
            packA_sb = pp.tile([128, FA], FP, name="packA_sb")
            nc.sync.dma_start(out=packA_sb[:], in_=packA_dr[:, :])
            off = 0
            for nm, rows, cols in PACK_A:
                wsb[nm] = packA_sb[0:rows, off:off + cols]
                off += cols

            ones_row = pp.tile([1, N], FP, name="ones_row")
            nc.vector.memset(ones_row[:], 1.0)
            ones_col = pp.tile([128, 1], FP, name="ones_col")
            nc.vector.memset(ones_col[:], 1.0)

            # feature tiles (channel-major), also the cat^T tiles for W5
            T0 = pp.tile([128, N], FP, name="catT0")  # x1 rows 0:64, x2 rows 64:128
            x2T = pp.tile([64, N], FP, name="x2T")    # base-0 copy of x2 for L3 matmuls
            T1 = pp.tile([128, N], FP, name="catT1")  # x3
            T2 = pp.tile([128, N], FP, name="catT2")  # x4[:128]
            T3 = pp.tile([128, N], FP, name="catT3")  # x4[128:]

            s_pm = pp.tile([128, NT], FP, name="s_pm")
            mbias = pp.tile([128, 1], FP, name="mbias")
            nc.vector.memset(mbias[:], float(mask_bias))
            mstar = pp.tile([128, NT], FP, name="mstar")
            mask_pm = pp.tile([128, NT], FP, name="mask_pm")
            mask_row = pp.tile([1, N], FP, name="mask_row")
            g_tiles = pp.tile([128, 16], FP, name="g_tiles")

            def layer(lp, C, O, Op, xT, wP, wQ, bQ, p_dr, write_out, l1=False):
                """one edgeconv layer. xT: AP [C, N]. write_out(t, z_sb): consume z."""
                # --- s row, -s/2 row ---
                xsq = lp.tile([C, N], FP, tag="xsq", bufs=2, name=f"xsq{O}_{C}")
                for cc in range(4):
                    csl = slice(cc * 512, (cc + 1) * 512)
                    nc.vector.tensor_tensor(out=xsq[:, csl], in0=xT[:, csl],
                                            in1=xT[:, csl], op=ALU.mult)
                use_r = USE_F32R and not l1
                if use_r:
                    xtr = lp.tile([C, N], FPR, tag="xtr", bufs=1, name=f"xtr{O}_{C}")
                    nc.scalar.activation(xtr[:], xT, ACTF.Copy)
                else:
                    xtr = None
                negs = lp.tile([1, N], FP, tag="negs", bufs=1, name=f"negs{O}_{C}")
                if l1:
                    s_row = lp.tile([1, N], FP, tag="srow", name="s_row")
                for cc in range(4):
                    s_ps = psp.tile([1, 512], FP, tag="misc", bufs=1, name=f"s_ps{O}_{cc}")
                    nc.tensor.matmul(s_ps[:], lhsT=ones_col[0:C, 0:1],
                                     rhs=xsq[:, cc * 512:(cc + 1) * 512],
                                     start=True, stop=True)
                    nc.scalar.activation(negs[0:1, cc * 512:(cc + 1) * 512], s_ps[:],
                                         ACTF.Copy, scale=-0.5)
                    if l1:
                        nc.scalar.activation(s_row[0:1, cc * 512:(cc + 1) * 512], s_ps[:],
                                             ACTF.Copy)
                if l1:
                    # s per-point, point-major [128, NT] (via DRAM bounce)
                    sw = nc.sync.dma_start(out=sdram[:, :], in_=s_row[:])
                    sr = nc.sync.dma_start(
                        out=s_pm[:],
                        in_=sdram[0:1, :].rearrange("one (t p) -> p (one t)", p=128))
                    tile.add_dep_helper(sr.ins, sw.ins, reason="sdram raw")

                # --- p (point-major) -> DRAM ---
                p_stage = lp.tile([128, NT, Op], FP, tag="pstage", name=f"pstage{O}_{C}")
                p_wr = []
                for t in range(NT):
                    p_ps = psp.tile([128, Op], FP, tag="pq", bufs=3, name=f"p_ps{O}_{t}")
                    nc.tensor.matmul(p_ps[:], lhsT=xT[:, t * 128:(t + 1) * 128],
                                     rhs=wP[0:C, 0:Op], start=True, stop=True)
                    nc.scalar.activation(p_stage[:, t, :], p_ps[:], ACTF.Copy)
                wr = nc.sync.dma_start(
                    out=p_dr[:, :].rearrange("(t p) o -> p t o", p=128),
                    in_=p_stage[:])
                p_wr.append(wr)

                # --- q (point-major, bias folded) ---
                q_pm = lp.tile([128, NT, 70 if l1 else O], FP, tag="qpm", name=f"qpm{O}_{C}")
                qw = 70 if l1 else O
                for t in range(NT):
                    q_ps = psp.tile([128, qw], FP, tag="pq", bufs=3, name=f"q_ps{O}_{t}")
                    nc.tensor.matmul(q_ps[:], lhsT=xT[:, t * 128:(t + 1) * 128],
                                     rhs=wQ[0:C, 0:qw], start=True, stop=False)
                    nc.tensor.matmul(q_ps[:], lhsT=ones_row[0:1, t * 128:(t + 1) * 128],
                                     rhs=bQ[0:1, 0:qw], start=False, stop=True)
                    nc.scalar.activation(q_pm[:, t, 0:qw], q_ps[:], ACTF.Copy)

                # --- per point-tile: gram -> topk -> gather -> reduce ---
                for t in range(NT):
                    r_sb = lp.tile([128, N], FP, tag="rsb", bufs=2, name=f"rsb{O}_{t}")
                    for cc in range(4):
                        r_ps = psp.tile([128, 512], FP, tag="rps", bufs=2,
                                        name=f"r_ps{O}_{t}_{cc}")
                        nc.tensor.matmul(r_ps[:], lhsT=xT[:, t * 128:(t + 1) * 128],
                                         rhs=xT[:, cc * 512:(cc + 1) * 512],
                                         start=True, stop=False)
                        nc.tensor.matmul(r_ps[:], lhsT=ones_row[0:1, 0:128],
                                         rhs=negs[0:1, cc * 512:(cc + 1) * 512],
                                         start=False, stop=True)
                        nc.scalar.activation(r_sb[:, cc * 512:(cc + 1) * 512], r_ps[:],
                                             ACTF.Copy)
                    if debug and l1 and t == 0:
                        nc.sync.dma_start(out=dbg["d_r1"][:, :], in_=r_sb[:])

                    vals = lp.tile([128, 24], FP, tag="vals", bufs=2, name=f"vals{O}_{t}")
                    idx = lp.tile([128, 24], U16, tag="idx", bufs=2, name=f"idx{O}_{t}")
                    for rnd in range(3):
                        sl = slice(rnd * 8, rnd * 8 + 8)
                        nc.vector.max(vals[:, sl], r_sb[:])
                        nc.vector.max_index(idx[:, sl], vals[:, sl], r_sb[:])
                        if rnd < 2:
                            nc.vector.match_replace(r_sb[:], vals[:, sl], r_sb[:], NEG_BIG)
                    if debug and l1 and t == 0:
                        nc.sync.dma_start(out=dbg["d_idx1"][:, :], in_=idx[:])

                    gth = lp.tile([128, K, Op], FP, tag=("gthB" if O == 256 else "gthA"),
                                  name=f"gth{O}_{t}")
                    # idx -> DRAM bounce -> wrapped-16 layout for dma_gather
                    iw = nc.sync.dma_start(
                        out=idxd[t * 128:(t + 1) * 128, :], in_=idx[:, 0:K])
                    wrap = lp.tile([128, 8 * K], U16, tag="wrap", bufs=2,
                                   name=f"wrap{O}_{t}")
                    ir = nc.sync.dma_start(
                        out=wrap[0:16, :].rearrange("pl (k ph) -> pl k ph", k=K),
                        in_=idxd[t * 128:(t + 1) * 128, :]
                            .rearrange("(ph pl) k -> pl k ph", pl=16))
                    tile.add_dep_helper(ir.ins, iw.ins, reason="idxd raw")
                    nc.sync.dma_start(out=wrap[16:32, :], in_=wrap[0:16, :])
                    nc.gpsimd.memset(wrap[32:64, :], 0)
                    nc.gpsimd.memset(wrap[64:96, :], 0)
                    nc.gpsimd.memset(wrap[96:128, :], 0)
                    g_ins = nc.gpsimd.dma_gather(
                        out_ap=gth[:], in_ap=p_dr[:, :],
                        idxs_ap=wrap[:].bitcast(I16),
                        num_idxs=K * 128, num_idxs_reg=K * 128,
                        elem_size=Op, single_packet=False)
                    for w in p_wr:
                        tile.add_dep_helper(g_ins.ins, w.ins, reason="pdram raw")

                    gmax = lp.tile([128, O], FP, tag="gmax", bufs=2, name=f"gmax{O}_{t}")
                    nc.vector.tensor_reduce(
                        out=gmax[:], in_=gth[:, :, 0:O].rearrange("p k o -> p o k"),
                        axis=AXT.X, op=ALU.max)
                    z = lp.tile([128, O], FP, tag="zsb", bufs=2, name=f"z{O}_{t}")
                    nc.vector.tensor_tensor(out=z[:], in0=gmax[:], in1=q_pm[:, t, 0:O],
                                            op=ALU.add)
                    # lrelu = max(z, 0.2 z)
                    z2 = lp.tile([128, O], FP, tag="zsb2", bufs=2, name=f"z2{O}_{t}")
                    nc.vector.tensor_scalar_mul(z2[:], z[:], 0.2)
                    nc.vector.tensor_tensor(out=z[:], in0=z[:], in1=z2[:], op=ALU.max)
                    write_out(t, z)

                    if l1:
                        # eu = sqrt(relu(s_n - 2 r_topk))
                        eu = lp.tile([128, K], FP, tag="eu", bufs=2, name=f"eu{t}")
                        nc.vector.tensor_scalar(
                            out=eu[:], in0=vals[:, 0:K], scalar1=-2.0, scalar2=s_pm[:, t:t + 1],
                            op0=ALU.mult, op1=ALU.add)
                        nc.vector.tensor_scalar_max(eu[:], eu[:], 0.0)
                        nc.scalar.activation(eu[:], eu[:], ACTF.Sqrt)
                        # z1 = gth[:, :, 64:70] + tm_ctr + eu*w9   -> [128, K, 6]
                        z1 = lp.tile([128, K, 6], FP, tag="z1", bufs=2, name=f"z1_{t}")
                        nc.vector.tensor_tensor(
                            out=z1[:], in0=gth[:, :, 64:70],
                            in1=q_pm[:, t:t + 1, 64:70].to_broadcast([128, K, 6]),
                            op=ALU.add)
                        ew = lp.tile([128, K, 6], FP, tag="ew", bufs=2, name=f"ew{t}")
                        nc.vector.tensor_tensor(
                            out=ew[:],
                            in0=eu[:].unsqueeze(2).to_broadcast([128, K, 6]),
                            in1=wsb["w9pm"][:].unsqueeze(1).to_broadcast([128, K, 6]),
                            op=ALU.mult)
                        nc.vector.tensor_tensor(out=z1[:], in0=z1[:], in1=ew[:], op=ALU.add)
                        nc.vector.tensor_scalar_mul(ew[:], z1[:], 0.2)
                        nc.vector.tensor_tensor(out=z1[:], in0=z1[:], in1=ew[:], op=ALU.max)
                        # y2 = sum_c z1*wm2 ; mstar[:, t] = max_k
                        nc.vector.tensor_tensor(
                            out=z1[:], in0=z1[:],
                            in1=wsb["wm2pm"][:].unsqueeze(1).to_broadcast([128, K, 6]),
                            op=ALU.mult)
                        y2 = lp.tile([128, K], FP, tag="y2", bufs=2, name=f"y2_{t}")
                        nc.vector.tensor_reduce(out=y2[:], in_=z1[:], axis=AXT.X, op=ALU.add)
                        nc.vector.tensor_reduce(out=mstar[:, t:t + 1], in_=y2[:],
                                                axis=AXT.X, op=ALU.max)

            # ========== layers ==========
            with tc.tile_pool(name="work", bufs=1) as lp:
                xt0 = lp.tile([3, N], FP, tag="xsq", bufs=2, name="xt0")
                nc.sync.dma_start(out=xt0[:], in_=x_in[:, :])

                def mk_writer(dsts, O, extra=None):
                    # dsts: list of (tile, row_offset); O total cols of z
                    def w(t, z):
                        for i, (dst, ro) in enumerate(dsts):
                            rows = min(128, O - 128 * i)
                            zT = psp.tile([rows, 128], FP, tag="zT", bufs=2,
                                          name=f"zT{O}_{t}_{i}")
                            nc.tensor.transpose(
                                out=zT[:], in_=z[:, i * 128:i * 128 + rows],
                                identity=wsb["ident"][:])
                            nc.scalar.activation(dst[ro:ro + rows, t * 128:(t + 1) * 128],
                                                 zT[:], ACTF.Copy)
                            if extra is not None and i == 0:
                                nc.scalar.activation(
                                    extra[0:rows, t * 128:(t + 1) * 128],
                                    zT[:], ACTF.Copy)
                    return w

                layer(lp, 3, 64, 128, xt0[0:3, :], wsb["wP1"], wsb["wQ1"], wsb["bQ1"],
                      pdram[0], mk_writer([(T0, 0)], 64), l1=True)
                # finish mask: sigmoid(|a| mstar + b)
                nc.scalar.activation(mask_pm[:], mstar[:], ACTF.Sigmoid,
                                     scale=float(mask_scale), bias=mbias[:])
                mw = nc.sync.dma_start(
                    out=mdram[0:1, :].rearrange("one (t p) -> p (one t)", p=128),
                    in_=mask_pm[:])
                mr = nc.sync.dma_start(out=mask_row[:], in_=mdram[:, :])
                tile.add_dep_helper(mr.ins, mw.ins, reason="mdram raw")

                layer(lp, 64, 64, 64, T0[0:64, :], wsb["wP2"], wsb["wQ2"], wsb["bQ2"],
                      pdram[1], mk_writer([(T0, 64)], 64, extra=x2T))
                layer(lp, 64, 128, 128, x2T[0:64, :], wsb["wP3"], wsb["wQ3"], wsb["bQ3"],
                      pdram[2], mk_writer([(T1, 0)], 128))
                layer(lp, 128, 256, 256, T1[0:128, :], wsb["wP4"], wsb["wQ4"], wsb["bQ4"],
                      pdram[3], mk_writer([(T2, 0), (T3, 0)], 256))

            if debug:
                nc.sync.dma_start(out=dbg["d_x1T"][:, :], in_=T0[0:64, :])
                nc.sync.dma_start(out=dbg["d_x2T"][:, :], in_=T0[64:128, :])
                nc.sync.dma_start(out=dbg["d_x3T"][:, :], in_=T1[:])
                nc.sync.dma_start(out=dbg["d_x4Ta"][:, :], in_=T2[:])
                nc.sync.dma_start(out=dbg["d_x4Tb"][:, :], in_=T3[:])
                nc.sync.dma_start(out=dbg["d_mask"][:, :], in_=mask_pm[:])

            # ========== head ==========
            with tc.tile_pool(name="head", bufs=1) as hp:
                F16 = mybir.dt.float16
                packC_sb = hp.tile([128, FC], F16, name="packC_sb")
                nc.sync.dma_start(out=packC_sb[:], in_=packC_dr[:, :])
                off = 0
                for nm, rows, cols in PACK_C:
                    wsb[nm] = packC_sb[0:rows, off:off + cols]
                    off += cols
                wsb["W5p"] = wsb["W5p"].rearrange("p (k o) -> p k o", k=4)
                hL1c = wsb["hL1"].rearrange("p (j o) -> p j o", j=16)
                wsb["hL2"] = wsb["hL2"].rearrange("p (k o) -> p k o", k=4)
                wsb["hL3"] = wsb["hL3"].rearrange("p (k o) -> p k o", k=2)
                mask_bc = hp.tile([128, N], FP, name="mask_bc")
                mask_row_h = hp.tile([1, N], F16, name="mask_row_h")
                nc.scalar.activation(mask_row_h[:], mask_row[:], ACTF.Copy)
                # mask broadcast [128, N] via PE replicate
                for cc in range(4):
                    m_ps = psp.tile([128, 512], FP, tag="rps", bufs=2, name=f"m_ps{cc}")
                    nc.tensor.matmul(m_ps[:], lhsT=ones_row[0:1, 0:128],
                                     rhs=mask_row[0:1, cc * 512:(cc + 1) * 512],
                                     start=True, stop=True)
                    nc.scalar.activation(mask_bc[:, cc * 512:(cc + 1) * 512], m_ps[:],
                                         ACTF.Copy)
                Th = []
                for ti, Tt in enumerate((T0, T1, T2, T3)):
                    th = hp.tile([128, N], F16, name=f"Th{ti}")
                    nc.vector.tensor_tensor(out=th[:], in0=Tt[:], in1=mask_bc[:],
                                            op=ALU.mult)
                    Th.append(th)

                # h^T tiles: 8 x [128, N]; W5p rows: [c(512), o(1024)]
                for ht in range(8):
                    h_sb = hp.tile([128, N], FP, tag="hsb", bufs=2, name=f"h_sb{ht}")
                    for cc in range(4):
                        h_ps = psp.tile([128, 512], FP, tag="rps", bufs=2,
                                        name=f"h_ps{ht}_{cc}")
                        for kc, Tt in enumerate(Th):
                            nc.tensor.matmul(
                                h_ps[:],
                                lhsT=wsb["W5p"][:, kc, ht * 128:(ht + 1) * 128],
                                rhs=Tt[:, cc * 512:(cc + 1) * 512],
                                start=(kc == 0), stop=False)
                        nc.tensor.matmul(
                            h_ps[:],
                            lhsT=wsb["t5row"][0:1, ht * 128:(ht + 1) * 128],
                            rhs=mask_row_h[0:1, cc * 512:(cc + 1) * 512],
                            start=False, stop=True)
                        nc.scalar.activation(h_sb[:, cc * 512:(cc + 1) * 512], h_ps[:],
                                             ACTF.Relu)
                    nc.vector.tensor_reduce(out=g_tiles[:, ht:ht + 1], in_=h_sb[:],
                                            axis=AXT.X, op=ALU.max)
                    nc.vector.tensor_reduce(out=g_tiles[:, 8 + ht:9 + ht], in_=h_sb[:],
                                            axis=AXT.X, op=ALU.add)
                if debug:
                    nc.sync.dma_start(out=dbg["d_g"][:, :], in_=g_tiles[:])

                g_h = hp.tile([128, 16], F16, name="g_h")
                nc.scalar.activation(g_h[:, 0:8], g_tiles[:, 0:8], ACTF.Copy)
                nc.scalar.activation(g_h[:, 8:16], g_tiles[:, 8:16], ACTF.Copy,
                                     scale=1.0 / 2048.0)
                g1_ps = psp.tile([1, 512], FP, tag="misc", bufs=1, name="g1_ps")
                for j in range(16):
                    nc.tensor.matmul(g1_ps[:], lhsT=g_h[:, j:j + 1],
                                     rhs=hL1c[:, j, :], start=(j == 0), stop=(j == 15))
                g1 = hp.tile([1, 512], FP, tag="g1", name="g1")
                nc.vector.tensor_tensor(out=g1[:], in0=g1_ps[:], in1=wsb["t6row"][:],
                                        op=ALU.add)
                g1b = hp.tile([1, 512], FP, tag="g1b", name="g1b")
                nc.vector.tensor_scalar_mul(g1b[:], g1[:], 0.2)
                nc.vector.tensor_tensor(out=g1[:], in0=g1[:], in1=g1b[:], op=ALU.max)
                g1T_ps = psp.tile([128, 4], FP, tag="zT", bufs=2, name="g1T_ps")
                for j in range(4):
                    nc.tensor.transpose(out=g1T_ps[:, j:j + 1],
                                        in_=g1[0:1, j * 128:(j + 1) * 128],
                                        identity=wsb["ident"][0:1, 0:1])
                g1T = hp.tile([128, 4], F16, tag="g1T", name="g1T")
                nc.scalar.activation(g1T[:], g1T_ps[:], ACTF.Copy)

                g2_ps = psp.tile([1, 256], FP, tag="misc", bufs=1, name="g2_ps")
                for j in range(4):
                    nc.tensor.matmul(g2_ps[:], lhsT=g1T[:, j:j + 1],
                                     rhs=wsb["hL2"][:, j, :],
                                     start=(j == 0), stop=(j == 3))
                g2 = hp.tile([1, 256], FP, tag="g2", name="g2")
                nc.vector.tensor_tensor(out=g2[:], in0=g2_ps[:], in1=wsb["bL2"][:],
                                        op=ALU.add)
                g2b = hp.tile([1, 256], FP, tag="g2b", name="g2b")
                nc.vector.tensor_scalar_mul(g2b[:], g2[:], 0.2)
                nc.vector.tensor_tensor(out=g2[:], in0=g2[:], in1=g2b[:], op=ALU.max)
                g2T_ps = psp.tile([128, 2], FP, tag="zT", bufs=2, name="g2T_ps")
                for j in range(2):
                    nc.tensor.transpose(out=g2T_ps[:, j:j + 1],
                                        in_=g2[0:1, j * 128:(j + 1) * 128],
                                        identity=wsb["ident"][0:1, 0:1])
                g2T = hp.tile([128, 2], F16, tag="g2T", name="g2T")
                nc.scalar.activation(g2T[:], g2T_ps[:], ACTF.Copy)

                o_ps = psp.tile([1, 40], FP, tag="misc", bufs=1, name="o_ps")
                for j in range(2):
                    nc.tensor.matmul(o_ps[:], lhsT=g2T[:, j:j + 1],
                                     rhs=wsb["hL3"][:, j, :],
                                     start=(j == 0), stop=(j == 1))
                o_sb = hp.tile([1, 40], FP, tag="osb", name="o_sb")
                nc.vector.tensor_tensor(out=o_sb[:], in0=o_ps[:], in1=wsb["bL3"][:],
                                        op=ALU.add)
                nc.sync.dma_start(out=out_dram[:, :], in_=o_sb[:])

    nc.compile()
    return nc


# --------------------------------------------------------------------------
# entry point
# --------------------------------------------------------------------------
def kernel(x, params):
    from concourse import bass_utils

    x = np.asarray(x)
    B = x.shape[0]
    w = prepare_weights(params)
    scal = (w.pop("_mask_scale"), w.pop("_mask_bias"))

    key = ("prog", scal)
    if key not in _CACHE:
        _CACHE[key] = build_program(*scal)
    nc = _CACHE[key]

    in_maps = []
    for b in range(B):
        m = {"x": np.ascontiguousarray(x[b], np.float32)}
        m.update(w)
        in_maps.append(m)

    res = bass_utils.run_bass_kernel_spmd(nc, in_maps, core_ids=list(range(B)))
    out = np.stack([res.results[b]["out"][0] for b in range(B)], axis=0)
    return out.astype(np.float32)


if __name__ == "__main__":
    pass


# revision 45
# speedup vs baseline: 1.1096x; 1.1096x over previous
"""ATMASKDGCNN Trainium2 kernel.

Data-parallel over batch: 8 samples -> 8 NeuronCores, one sample per core.
Each core runs an identical Bass program on its own sample; no collectives.

Math (validated vs reference in fp64):
  EdgeConv(x; W, bn) = lrelu(max_k p[idx[n,k]] + q[n])
    with p = x @ (s*Wd)^T, q = x @ (s*(Wc-Wd))^T + (beta - s*mean),
    Wd = W[:, :C], Wc = W[:, C:], s = gamma*rsqrt(var+eps).
  KNN ranking by r[n,m] = x_n . x_m - |x_m|^2/2  (row-monotone equiv of -d^2/2)
  pd[n,m] = 2*r[n,m] - |x_n|^2  (used for eu in the L1 mask branch).
  mask  = sigmoid(|a| * max_k(sign(a)*y2) + b)  (bn2m affine folded; sign in wm2)
  h     = mask * relu(bn5(cat @ W5^T))   (relu(lrelu(z)) == relu(z); mask>0)
  head: plain affine-folded MLP per sample.
"""

import os
import numpy as np

N = 2048
K = 20
NT = N // 128  # 16 point-tiles
BN_EPS = 1e-5
NEG_BIG = -3.0e38
USE_F32R = False

_CACHE = {}


# --------------------------------------------------------------------------
# host-side weight preprocessing
# --------------------------------------------------------------------------
def _bn_affine(p):
    g, b, m, v = [np.asarray(t, np.float64) for t in p]
    s = g / np.sqrt(v + BN_EPS)
    return s, b - s * m


def prepare_weights(params):
    f32 = lambda a: np.ascontiguousarray(a, np.float32)
    W = {k: np.asarray(v, np.float64) for k, v in params.items() if not isinstance(v, tuple)}
    s1, t1 = _bn_affine(params["bn1"]); s2, t2 = _bn_affine(params["bn2"])
    s3, t3 = _bn_affine(params["bn3"]); s4, t4 = _bn_affine(params["bn4"])
    s5, t5 = _bn_affine(params["bn5"]); sm1, tm1 = _bn_affine(params["bnm1"])
    sm2, tm2 = _bn_affine(params["bnm2"]); s6, t6 = _bn_affine(params["bn6"])
    s7, t7 = _bn_affine(params["bn7"])

    out = {}
    # L1: p rows [p0(64) | tm_nb(6) | pad(2)]
    W1 = W["W1"]; Wm1 = W["Wm1"]
    P1 = np.zeros((3, 128)); P1[:, :64] = (s1[:, None] * W1[:, :3]).T
    P1[:, 64:70] = (sm1[:, None] * (Wm1[:, 0:3] + Wm1[:, 6:9])).T
    out["wP1"] = f32(P1)
    Q1 = np.zeros((3, 70)); Q1[:, :64] = (s1[:, None] * (W1[:, 3:6] - W1[:, :3])).T
    Q1[:, 64:70] = (sm1[:, None] * (Wm1[:, 3:6] - Wm1[:, 0:3])).T
    out["wQ1"] = f32(Q1)
    B1 = np.zeros((1, 70)); B1[0, :64] = t1; B1[0, 64:70] = tm1
    out["bQ1"] = f32(B1)

    for i, (Wk, sk, tk, C) in enumerate(
        [("W2", s2, t2, 64), ("W3", s3, t3, 64), ("W4", s4, t4, 128)], start=2
    ):
        Wi = W[Wk]
        out[f"wP{i}"] = f32((sk[:, None] * Wi[:, :C]).T)
        out[f"wQ{i}"] = f32((sk[:, None] * (Wi[:, C:] - Wi[:, :C])).T)
        out[f"bQ{i}"] = f32(tk[None, :])

    # mask branch constants
    a = sm2[0]; bm = tm2[0]; sgn = 1.0 if a >= 0 else -1.0
    out["w9pm"] = f32(np.broadcast_to((sm1 * Wm1[:, 9])[None, :], (128, 6)))
    out["wm2pm"] = f32(np.broadcast_to((sgn * W["Wm2"][0])[None, :], (128, 6)))
    out["_mask_scale"] = float(abs(a)); out["_mask_bias"] = float(bm)

    # W5 (s5 folded), bias via mask-row matmul
    out["W5p"] = f32((s5[:, None] * W["W5"]).T)        # [512, 1024]
    out["t5row"] = f32(t5[None, :])                    # [1, 1024]

    # head L1: g layout col j*128+p: j<8 -> max of h[j*128+p], j>=8 -> sum/2048
    L1 = W["L1"]  # [512, 2048]
    hL1 = np.empty((2048, 512))
    for j in range(16):
        for_cols = np.arange(128) + (j * 128 if j < 8 else 1024 + (j - 8) * 128)
        blk = L1[:, for_cols].T * s6[None, :]
        hL1[j * 128:(j + 1) * 128] = blk
    out["hL1"] = f32(hL1)
    out["t6row"] = f32(t6[None, :])
    out["hL2"] = f32((s7[:, None] * W["L2"]).T)        # [512, 256]
    out["bL2"] = f32((s7 * W["b2"] + t7)[None, :])
    out["hL3"] = f32(W["L3"].T)                        # [256, 40]
    out["bL3"] = f32(W["b3"][None, :])
    out["ident"] = f32(np.eye(128))
    return pack_weights(out)


# pack layout tables: (name, rows, cols). Folded entries in B are pre-folded
# host-side into [128, cols].
PACK_A = [
    ("wP1", 3, 128), ("wQ1", 3, 70), ("bQ1", 1, 70),
    ("wP2", 64, 64), ("wQ2", 64, 64), ("bQ2", 1, 64),
    ("wP3", 64, 128), ("wQ3", 64, 128), ("bQ3", 1, 128),
    ("wP4", 128, 256), ("wQ4", 128, 256), ("bQ4", 1, 256),
    ("w9pm", 128, 6), ("wm2pm", 128, 6), ("ident", 128, 128),
]
PACK_A += [("t6row", 1, 512), ("bL2", 1, 256), ("bL3", 1, 40)]
# fp16 head weights, folded host-side into [128, cols]
PACK_C = [
    ("W5p", 128, 4096), ("hL1", 128, 8192), ("hL2", 128, 1024),
    ("hL3", 128, 80), ("t5row", 1, 1024),
]
FA = sum(c for _, _, c in PACK_A)
FC = sum(c for _, _, c in PACK_C)


def _fold(a, p=128):
    k = a.shape[0] // p
    return a.reshape(k, p, a.shape[1]).transpose(1, 0, 2).reshape(p, -1)


def pack_weights(w):
    packA = np.zeros((128, FA), np.float32)
    off = 0
    for nm, rows, cols in PACK_A:
        packA[0:rows, off:off + cols] = w[nm]
        off += cols
    packC = np.zeros((128, FC), np.float16)
    off = 0
    for nm, rows, cols in PACK_C:
        a = w[nm]
        if a.shape[0] > 128:
            a = _fold(a)
        packC[0:rows, off:off + cols] = a.reshape(rows, cols).astype(np.float16)
        off += cols
    return {"wpackA": packA, "wpackC": packC,
            "_mask_scale": w["_mask_scale"], "_mask_bias": w["_mask_bias"]}


# --------------------------------------------------------------------------
# device program
# --------------------------------------------------------------------------
def build_program(mask_scale, mask_bias, debug=False):
    import concourse.bass as bass
    import concourse.bacc as bacc
    import concourse.mybir as mybir
    import concourse.tile as tile
    from concourse.bass import IndirectOffsetOnAxis

    FP = mybir.dt.float32
    U32 = mybir.dt.uint32
    U16 = mybir.dt.uint16
    I16 = mybir.dt.int16
    ALU = mybir.AluOpType
    AXT = mybir.AxisListType
    ACTF = mybir.ActivationFunctionType

    nc = bacc.Bacc("TRN2", target_bir_lowering=False, debug=False)

    # ---- I/O ----
    x_in = nc.dram_tensor("x", [3, N], FP, kind="ExternalInput")
    packA_dr = nc.dram_tensor("wpackA", [128, FA], FP, kind="ExternalInput")
    packC_dr = nc.dram_tensor("wpackC", [128, FC], mybir.dt.float16, kind="ExternalInput")
    out_dram = nc.dram_tensor("out", [1, 40], FP, kind="ExternalOutput")

    sdram = nc.dram_tensor("sdram", [1, N], mybir.dt.float32, kind="Internal")
    idxd = nc.dram_tensor("idxd", [NT * 128, 20], mybir.dt.uint16, kind="Internal")
    mdram = nc.dram_tensor("mdram", [1, N], mybir.dt.float32, kind="Internal")
    pdram = [
        nc.dram_tensor("pd1", [N, 128], FP, kind="Internal"),
        nc.dram_tensor("pd2", [N, 64], FP, kind="Internal"),
        nc.dram_tensor("pd3", [N, 128], FP, kind="Internal"),
        nc.dram_tensor("pd4", [N, 256], FP, kind="Internal"),
    ]
    dbg = {}
    if debug:
        for nm, shp in [("d_x1T", (64, N)), ("d_x2T", (64, N)), ("d_x3T", (128, N)),
                        ("d_x4Ta", (128, N)), ("d_x4Tb", (128, N)),
                        ("d_mask", (128, 16)), ("d_g", (128, 16)),
                        ("d_idx1", (128, 24)), ("d_r1", (128, N))]:
            dbg[nm] = nc.dram_tensor(nm, list(shp), FP if not nm.startswith("d_idx") else mybir.dt.uint16,
                                     kind="ExternalOutput")

    with tile.TileContext(nc) as tc:
        with tc.tile_pool(name="persist", bufs=1) as pp, \
             tc.tile_pool(name="psum", bufs=1, space="PSUM") as psp:

            # ---- load layer weights (single packed DMA) ----
            wsb = {}
            packA_sb = pp.tile([128, FA], FP, name="packA_sb")
            nc.sync.dma_start(out=packA_sb[:], in_=packA_dr[:, :])
            off = 0
            for nm, rows, cols in PACK_A:
                wsb[nm] = packA_sb[0:rows, off:off + cols]
                off += cols

            ones_row = pp.tile([1, N], FP, name="ones_row")
            nc.vector.memset(ones_row[:], 1.0)
            iota_u32 = pp.tile([128, N], U32, name="iota_u32")
            nc.gpsimd.iota(iota_u32[:], pattern=[[1, N]], base=0, channel_multiplier=0)
            ones_col = pp.tile([128, 1], FP, name="ones_col")
            nc.vector.memset(ones_col[:], 1.0)

            # feature tiles (channel-major), also the cat^T tiles for W5
            T0 = pp.tile([128, N], FP, name="catT0")  # x1 rows 0:64, x2 rows 64:128
            x2T = pp.tile([64, N], FP, name="x2T")    # base-0 copy of x2 for L3 matmuls
            T1 = pp.tile([128, N], FP, name="catT1")  # x3
            T2 = pp.tile([128, N], FP, name="catT2")  # x4[:128]
            T3 = pp.tile([128, N], FP, name="catT3")  # x4[128:]

            s_pm = pp.tile([128, NT], FP, name="s_pm")
            mbias = pp.tile([128, 1], FP, name="mbias")
            nc.vector.memset(mbias[:], float(mask_bias))
            mstar = pp.tile([128, NT], FP, name="mstar")
            mask_pm = pp.tile([128, NT], FP, name="mask_pm")
            mask_row = pp.tile([1, N], FP, name="mask_row")
            g_tiles = pp.tile([128, 16], FP, name="g_tiles")

            def layer(lp, C, O, Op, xT, wP, wQ, bQ, p_dr, write_out, l1=False):
                """one edgeconv layer. xT: AP [C, N]. write_out(t, z_sb): consume z."""
                # --- s row, -s/2 row ---
                xsq = lp.tile([C, N], FP, tag="xsq", bufs=2, name=f"xsq{O}_{C}")
                for cc in range(4):
                    csl = slice(cc * 512, (cc + 1) * 512)
                    nc.vector.tensor_tensor(out=xsq[:, csl], in0=xT[:, csl],
                                            in1=xT[:, csl], op=ALU.mult)
                use_r = USE_F32R and not l1
                if use_r:
                    xtr = lp.tile([C, N], FPR, tag="xtr", bufs=1, name=f"xtr{O}_{C}")
                    nc.scalar.activation(xtr[:], xT, ACTF.Copy)
                else:
                    xtr = None
                negs = lp.tile([1, N], FP, tag="negs", bufs=1, name=f"negs{O}_{C}")
                for cc in range(4):
                    s_ps = psp.tile([1, 512], FP, tag="misc", bufs=1, name=f"s_ps{O}_{cc}")
                    nc.tensor.matmul(s_ps[:], lhsT=ones_col[0:C, 0:1],
                                     rhs=xsq[:, cc * 512:(cc + 1) * 512],
                                     start=True, stop=True)
                    nc.scalar.activation(negs[0:1, cc * 512:(cc + 1) * 512], s_ps[:],
                                         ACTF.Copy, scale=-0.5)
                # -s/2 per point, point-major [128, NT] (DRAM bounce) for the
                # recentered+index-packed topk rows
                s_pm2 = lp.tile([128, NT], FP, tag="spm2", bufs=2, name=f"spm2{O}_{C}")
                sw = nc.sync.dma_start(out=sdram[:, :], in_=negs[:])
                sr = nc.sync.dma_start(
                    out=s_pm2[:],
                    in_=sdram[0:1, :].rearrange("one (t p) -> p (one t)", p=128))
                tile.add_dep_helper(sr.ins, sw.ins, reason="sdram raw")

                # --- p (point-major) -> DRAM ---
                p_stage = lp.tile([128, NT, Op], FP, tag="pstage", name=f"pstage{O}_{C}")
                p_wr = []
                for t in range(NT):
                    p_ps = psp.tile([128, Op], FP, tag="pq", bufs=3, name=f"p_ps{O}_{t}")
                    nc.tensor.matmul(p_ps[:], lhsT=xT[:, t * 128:(t + 1) * 128],
                                     rhs=wP[0:C, 0:Op], start=True, stop=True)
                    nc.scalar.activation(p_stage[:, t, :], p_ps[:], ACTF.Copy)
                wr = nc.sync.dma_start(
                    out=p_dr[:, :].rearrange("(t p) o -> p t o", p=128),
                    in_=p_stage[:])
                p_wr.append(wr)

                # --- q (point-major, bias folded) ---
                q_pm = lp.tile([128, NT, 70 if l1 else O], FP, tag="qpm", name=f"qpm{O}_{C}")
                qw = 70 if l1 else O
                for t in range(NT):
                    q_ps = psp.tile([128, qw], FP, tag="pq", bufs=3, name=f"q_ps{O}_{t}")
                    nc.tensor.matmul(q_ps[:], lhsT=xT[:, t * 128:(t + 1) * 128],
                                     rhs=wQ[0:C, 0:qw], start=True, stop=False)
                    nc.tensor.matmul(q_ps[:], lhsT=ones_row[0:1, t * 128:(t + 1) * 128],
                                     rhs=bQ[0:1, 0:qw], start=False, stop=True)
                    nc.scalar.activation(q_pm[:, t, 0:qw], q_ps[:], ACTF.Copy)

                # --- per point-tile: gram -> topk -> gather -> reduce ---
                for t in range(NT):
                    r_sb = lp.tile([128, N], FP, tag="rsb", bufs=2, name=f"rsb{O}_{t}")
                    for cc in range(4):
                        r_ps = psp.tile([128, 512], FP, tag="rps", bufs=2,
                                        name=f"r_ps{O}_{t}_{cc}")
                        nc.tensor.matmul(r_ps[:], lhsT=xT[:, t * 128:(t + 1) * 128],
                                         rhs=xT[:, cc * 512:(cc + 1) * 512],
                                         start=True, stop=False)
                        nc.tensor.matmul(r_ps[:], lhsT=ones_row[0:1, 0:128],
                                         rhs=negs[0:1, cc * 512:(cc + 1) * 512],
                                         start=False, stop=True)
                        nc.scalar.activation(r_sb[:, cc * 512:(cc + 1) * 512], r_ps[:],
                                             ACTF.Identity, bias=s_pm2[:, t:t + 1])
                    # pack column index into low 11 mantissa bits (rows are
                    # recentered to pd/2, so truncation << neighbor gaps)
                    rv = r_sb[:].bitcast(U32)
                    nc.vector.tensor_scalar(out=rv, in0=rv, scalar1=0xFFFFF800,
                                            scalar2=None, op0=ALU.bitwise_and)
                    nc.vector.tensor_tensor(out=rv, in0=rv, in1=iota_u32[:],
                                            op=ALU.bitwise_or)
                    if debug and l1 and t == 0:
                        nc.sync.dma_start(out=dbg["d_r1"][:, :], in_=r_sb[:])

                    vals = lp.tile([128, 24], FP, tag="vals", bufs=2, name=f"vals{O}_{t}")
                    idxw = lp.tile([128, 24], U32, tag="idxw", bufs=2, name=f"idxw{O}_{t}")
                    idx = lp.tile([128, 24], U16, tag="idx", bufs=2, name=f"idx{O}_{t}")
                    for rnd in range(3):
                        sl = slice(rnd * 8, rnd * 8 + 8)
                        nc.vector.max(vals[:, sl], r_sb[:])
                        if rnd < 2:
                            nc.vector.match_replace(r_sb[:], vals[:, sl], r_sb[:], NEG_BIG)
                    nc.vector.tensor_scalar(out=idxw[:], in0=vals[:].bitcast(U32),
                                            scalar1=0x7FF, scalar2=None,
                                            op0=ALU.bitwise_and)
                    nc.vector.tensor_copy(idx[:], idxw[:])
                    if l1:
                        nc.vector.tensor_scalar(out=vals[:].bitcast(U32),
                                                in0=vals[:].bitcast(U32),
                                                scalar1=0xFFFFF800, scalar2=None,
                                                op0=ALU.bitwise_and)
                    if debug and l1 and t == 0:
                        nc.sync.dma_start(out=dbg["d_idx1"][:, :], in_=idx[:])

                    gth = lp.tile([128, K, Op], FP, tag=("gthB" if O == 256 else "gthA"),
                                  name=f"gth{O}_{t}")
                    # idx -> DRAM bounce -> wrapped-16 layout for dma_gather
                    iw = nc.sync.dma_start(
                        out=idxd[t * 128:(t + 1) * 128, :], in_=idx[:, 0:K])
                    wrap = lp.tile([128, 8 * K], U16, tag="wrap", bufs=2,
                                   name=f"wrap{O}_{t}")
                    ir = nc.sync.dma_start(
                        out=wrap[0:16, :].rearrange("pl (k ph) -> pl k ph", k=K),
                        in_=idxd[t * 128:(t + 1) * 128, :]
                            .rearrange("(ph pl) k -> pl k ph", pl=16))
                    tile.add_dep_helper(ir.ins, iw.ins, reason="idxd raw")
                    nc.sync.dma_start(out=wrap[16:32, :], in_=wrap[0:16, :])
                    nc.gpsimd.memset(wrap[32:64, :], 0)
                    nc.gpsimd.memset(wrap[64:96, :], 0)
                    nc.gpsimd.memset(wrap[96:128, :], 0)
                    g_ins = nc.gpsimd.dma_gather(
                        out_ap=gth[:], in_ap=p_dr[:, :],
                        idxs_ap=wrap[:].bitcast(I16),
                        num_idxs=K * 128, num_idxs_reg=K * 128,
                        elem_size=Op, single_packet=False)
                    for w in p_wr:
                        tile.add_dep_helper(g_ins.ins, w.ins, reason="pdram raw")

                    gmax = lp.tile([128, O], FP, tag="gmax", bufs=2, name=f"gmax{O}_{t}")
                    nc.vector.tensor_reduce(
                        out=gmax[:], in_=gth[:, :, 0:O].rearrange("p k o -> p o k"),
                        axis=AXT.X, op=ALU.max)
                    z = lp.tile([128, O], FP, tag="zsb", bufs=2, name=f"z{O}_{t}")
                    nc.vector.tensor_tensor(out=z[:], in0=gmax[:], in1=q_pm[:, t, 0:O],
                                            op=ALU.add)
                    # lrelu = max(z, 0.2 z)
                    z2 = lp.tile([128, O], FP, tag="zsb2", bufs=2, name=f"z2{O}_{t}")
                    nc.vector.tensor_scalar_mul(z2[:], z[:], 0.2)
                    nc.vector.tensor_tensor(out=z[:], in0=z[:], in1=z2[:], op=ALU.max)
                    write_out(t, z)

                    if l1:
                        # eu = sqrt(relu(s_n - 2 r_topk))
                        eu = lp.tile([128, K], FP, tag="eu", bufs=2, name=f"eu{t}")
                        nc.vector.tensor_scalar_mul(eu[:], vals[:, 0:K], -2.0)
                        nc.vector.tensor_scalar_max(eu[:], eu[:], 0.0)
                        nc.scalar.activation(eu[:], eu[:], ACTF.Sqrt)
                        # z1 = gth[:, :, 64:70] + tm_ctr + eu*w9   -> [128, K, 6]
                        z1 = lp.tile([128, K, 6], FP, tag="z1", bufs=2, name=f"z1_{t}")
                        nc.vector.tensor_tensor(
                            out=z1[:], in0=gth[:, :, 64:70],
                            in1=q_pm[:, t:t + 1, 64:70].to_broadcast([128, K, 6]),
                            op=ALU.add)
                        ew = lp.tile([128, K, 6], FP, tag="ew", bufs=2, name=f"ew{t}")
                        nc.vector.tensor_tensor(
                            out=ew[:],
                            in0=eu[:].unsqueeze(2).to_broadcast([128, K, 6]),
                            in1=wsb["w9pm"][:].unsqueeze(1).to_broadcast([128, K, 6]),
                            op=ALU.mult)
                        nc.vector.tensor_tensor(out=z1[:], in0=z1[:], in1=ew[:], op=ALU.add)
                        nc.vector.tensor_scalar_mul(ew[:], z1[:], 0.2)
                        nc.vector.tensor_tensor(out=z1[:], in0=z1[:], in1=ew[:], op=ALU.max)
                        # y2 = sum_c z1*wm2 ; mstar[:, t] = max_k
                        nc.vector.tensor_tensor(
                            out=z1[:], in0=z1[:],
                            in1=wsb["wm2pm"][:].unsqueeze(1).to_broadcast([128, K, 6]),
                            op=ALU.mult)
                        y2 = lp.tile([128, K], FP, tag="y2", bufs=2, name=f"y2_{t}")
                        nc.vector.tensor_reduce(out=y2[:], in_=z1[:], axis=AXT.X, op=ALU.add)
                        nc.vector.tensor_reduce(out=mstar[:, t:t + 1], in_=y2[:],
                                                axis=AXT.X, op=ALU.max)

            # ========== layers ==========
            with tc.tile_pool(name="work", bufs=1) as lp:
                xt0 = lp.tile([3, N], FP, tag="xsq", bufs=2, name="xt0")
                nc.sync.dma_start(out=xt0[:], in_=x_in[:, :])

                def mk_writer(dsts, O, extra=None):
                    # dsts: list of (tile, row_offset); O total cols of z
                    def w(t, z):
                        for i, (dst, ro) in enumerate(dsts):
                            rows = min(128, O - 128 * i)
                            zT = psp.tile([rows, 128], FP, tag="zT", bufs=2,
                                          name=f"zT{O}_{t}_{i}")
                            nc.tensor.transpose(
                                out=zT[:], in_=z[:, i * 128:i * 128 + rows],
                                identity=wsb["ident"][:])
                            nc.scalar.activation(dst[ro:ro + rows, t * 128:(t + 1) * 128],
                                                 zT[:], ACTF.Copy)
                            if extra is not None and i == 0:
                                nc.scalar.activation(
                                    extra[0:rows, t * 128:(t + 1) * 128],
                                    zT[:], ACTF.Copy)
                    return w

                layer(lp, 3, 64, 128, xt0[0:3, :], wsb["wP1"], wsb["wQ1"], wsb["bQ1"],
                      pdram[0], mk_writer([(T0, 0)], 64), l1=True)
                # finish mask: sigmoid(|a| mstar + b)
                nc.scalar.activation(mask_pm[:], mstar[:], ACTF.Sigmoid,
                                     scale=float(mask_scale), bias=mbias[:])
                mw = nc.sync.dma_start(
                    out=mdram[0:1, :].rearrange("one (t p) -> p (one t)", p=128),
                    in_=mask_pm[:])
                mr = nc.sync.dma_start(out=mask_row[:], in_=mdram[:, :])
                tile.add_dep_helper(mr.ins, mw.ins, reason="mdram raw")

                layer(lp, 64, 64, 64, T0[0:64, :], wsb["wP2"], wsb["wQ2"], wsb["bQ2"],
                      pdram[1], mk_writer([(T0, 64)], 64, extra=x2T))
                layer(lp, 64, 128, 128, x2T[0:64, :], wsb["wP3"], wsb["wQ3"], wsb["bQ3"],
                      pdram[2], mk_writer([(T1, 0)], 128))
                layer(lp, 128, 256, 256, T1[0:128, :], wsb["wP4"], wsb["wQ4"], wsb["bQ4"],
                      pdram[3], mk_writer([(T2, 0), (T3, 0)], 256))

            if debug:
                nc.sync.dma_start(out=dbg["d_x1T"][:, :], in_=T0[0:64, :])
                nc.sync.dma_start(out=dbg["d_x2T"][:, :], in_=T0[64:128, :])
                nc.sync.dma_start(out=dbg["d_x3T"][:, :], in_=T1[:])
                nc.sync.dma_start(out=dbg["d_x4Ta"][:, :], in_=T2[:])
                nc.sync.dma_start(out=dbg["d_x4Tb"][:, :], in_=T3[:])
                nc.sync.dma_start(out=dbg["d_mask"][:, :], in_=mask_pm[:])

            # ========== head ==========
            with tc.tile_pool(name="head", bufs=1) as hp:
                F16 = mybir.dt.float16
                packC_sb = hp.tile([128, FC], F16, name="packC_sb")
                nc.sync.dma_start(out=packC_sb[:], in_=packC_dr[:, :])
                off = 0
                for nm, rows, cols in PACK_C:
                    wsb[nm] = packC_sb[0:rows, off:off + cols]
                    off += cols
                wsb["W5p"] = wsb["W5p"].rearrange("p (k o) -> p k o", k=4)
                hL1c = wsb["hL1"].rearrange("p (j o) -> p j o", j=16)
                wsb["hL2"] = wsb["hL2"].rearrange("p (k o) -> p k o", k=4)
                wsb["hL3"] = wsb["hL3"].rearrange("p (k o) -> p k o", k=2)
                mask_bc = hp.tile([128, N], FP, name="mask_bc")
                mask_row_h = hp.tile([1, N], F16, name="mask_row_h")
                nc.scalar.activation(mask_row_h[:], mask_row[:], ACTF.Copy)
                # mask broadcast [128, N] via PE replicate
                for cc in range(4):
                    m_ps = psp.tile([128, 512], FP, tag="rps", bufs=2, name=f"m_ps{cc}")
                    nc.tensor.matmul(m_ps[:], lhsT=ones_row[0:1, 0:128],
                                     rhs=mask_row[0:1, cc * 512:(cc + 1) * 512],
                                     start=True, stop=True)
                    nc.scalar.activation(mask_bc[:, cc * 512:(cc + 1) * 512], m_ps[:],
                                         ACTF.Copy)
                Th = []
                for ti, Tt in enumerate((T0, T1, T2, T3)):
                    th = hp.tile([128, N], F16, name=f"Th{ti}")
                    nc.vector.tensor_tensor(out=th[:], in0=Tt[:], in1=mask_bc[:],
                                            op=ALU.mult)
                    Th.append(th)

                # h^T tiles: 8 x [128, N]; W5p rows: [c(512), o(1024)]
                for ht in range(8):
                    h_sb = hp.tile([128, N], FP, tag="hsb", bufs=2, name=f"h_sb{ht}")
                    for cc in range(4):
                        h_ps = psp.tile([128, 512], FP, tag="rps", bufs=2,
                                        name=f"h_ps{ht}_{cc}")
                        for kc, Tt in enumerate(Th):
                            nc.tensor.matmul(
                                h_ps[:],
                                lhsT=wsb["W5p"][:, kc, ht * 128:(ht + 1) * 128],
                                rhs=Tt[:, cc * 512:(cc + 1) * 512],
                                start=(kc == 0), stop=False)
                        nc.tensor.matmul(
                            h_ps[:],
                            lhsT=wsb["t5row"][0:1, ht * 128:(ht + 1) * 128],
                            rhs=mask_row_h[0:1, cc * 512:(cc + 1) * 512],
                            start=False, stop=True)
                        nc.scalar.activation(h_sb[:, cc * 512:(cc + 1) * 512], h_ps[:],
                                             ACTF.Relu)
                    nc.vector.tensor_reduce(out=g_tiles[:, ht:ht + 1], in_=h_sb[:],
                                            axis=AXT.X, op=ALU.max)
                    nc.vector.tensor_reduce(out=g_tiles[:, 8 + ht:9 + ht], in_=h_sb[:],
                                            axis=AXT.X, op=ALU.add)
                if debug:
                    nc.sync.dma_start(out=dbg["d_g"][:, :], in_=g_tiles[:])

                g_h = hp.tile([128, 16], F16, name="g_h")
                nc.scalar.activation(g_h[:, 0:8], g_tiles[:, 0:8], ACTF.Copy)
                nc.scalar.activation(g_h[:, 8:16], g_tiles[:, 8:16], ACTF.Copy,
                                     scale=1.0 / 2048.0)
                g1_ps = psp.tile([1, 512], FP, tag="misc", bufs=1, name="g1_ps")
                for j in range(16):
                    nc.tensor.matmul(g1_ps[:], lhsT=g_h[:, j:j + 1],
                                     rhs=hL1c[:, j, :], start=(j == 0), stop=(j == 15))
                g1 = hp.tile([1, 512], FP, tag="g1", name="g1")
                nc.vector.tensor_tensor(out=g1[:], in0=g1_ps[:], in1=wsb["t6row"][:],
                                        op=ALU.add)
                g1b = hp.tile([1, 512], FP, tag="g1b", name="g1b")
                nc.vector.tensor_scalar_mul(g1b[:], g1[:], 0.2)
                nc.vector.tensor_tensor(out=g1[:], in0=g1[:], in1=g1b[:], op=ALU.max)
                g1T_ps = psp.tile([128, 4], FP, tag="zT", bufs=2, name="g1T_ps")
                for j in range(4):
                    nc.tensor.transpose(out=g1T_ps[:, j:j + 1],
                                        in_=g1[0:1, j * 128:(j + 1) * 128],
                                        identity=wsb["ident"][0:1, 0:1])
                g1T = hp.tile([128, 4], F16, tag="g1T", name="g1T")
                nc.scalar.activation(g1T[:], g1T_ps[:], ACTF.Copy)

                g2_ps = psp.tile([1, 256], FP, tag="misc", bufs=1, name="g2_ps")
                for j in range(4):
                    nc.tensor.matmul(g2_ps[:], lhsT=g1T[:, j:j + 1],
                                     rhs=wsb["hL2"][:, j, :],
                                     start=(j == 0), stop=(j == 3))
                g2 = hp.tile([1, 256], FP, tag="g2", name="g2")
                nc.vector.tensor_tensor(out=g2[:], in0=g2_ps[:], in1=wsb["bL2"][:],
                                        op=ALU.add)
                g2b = hp.tile([1, 256], FP, tag="g2b", name="g2b")
                nc.vector.tensor_scalar_mul(g2b[:], g2[:], 0.2)
                nc.vector.tensor_tensor(out=g2[:], in0=g2[:], in1=g2b[:], op=ALU.max)
                g2T_ps = psp.tile([128, 2], FP, tag="zT", bufs=2, name="g2T_ps")
                for j in range(2):
                    nc.tensor.transpose(out=g2T_ps[:, j:j + 1],
                                        in_=g2[0:1, j * 128:(j + 1) * 128],
                                        identity=wsb["ident"][0:1, 0:1])
                g2T = hp.tile([128, 2], F16, tag="g2T", name="g2T")
                nc.scalar.activation(g2T[:], g2T_ps[:], ACTF.Copy)

                o_ps = psp.tile([1, 40], FP, tag="misc", bufs=1, name="o_ps")
                for j in range(2):
                    nc.tensor.matmul(o_ps[:], lhsT=g2T[:, j:j + 1],
                                     rhs=wsb["hL3"][:, j, :],
                                     start=(j == 0), stop=(j == 1))
                o_sb = hp.tile([1, 40], FP, tag="osb", name="o_sb")
                nc.vector.tensor_tensor(out=o_sb[:], in0=o_ps[:], in1=wsb["bL3"][:],
                                        op=ALU.add)
                nc.sync.dma_start(out=out_dram[:, :], in_=o_sb[:])

    nc.compile()
    return nc


# --------------------------------------------------------------------------
# entry point
# --------------------------------------------------------------------------
def kernel(x, params):
    from concourse import bass_utils

    x = np.asarray(x)
    B = x.shape[0]
    w = prepare_weights(params)
    scal = (w.pop("_mask_scale"), w.pop("_mask_bias"))

    key = ("prog", scal)
    if key not in _CACHE:
        _CACHE[key] = build_program(*scal)
    nc = _CACHE[key]

    in_maps = []
    for b in range(B):
        m = {"x": np.ascontiguousarray(x[b], np.float32)}
        m.update(w)
        in_maps.append(m)

    res = bass_utils.run_bass_kernel_spmd(nc, in_maps, core_ids=list(range(B)))
    out = np.stack([res.results[b]["out"][0] for b in range(B)], axis=0)
    return out.astype(np.float32)


if __name__ == "__main__":
    pass
